# revision 4
# baseline (speedup 1.0000x reference)
# Trainium2 Bass kernel for EndPointRepr (span endpoint representations).
#
# reference:
#   h = encoded_input @ W + b                    # [B, S, P]
#   res_k[q] = concat(h[qb[q], s_k[q]], h[qb[q], e_k[q]]) * (e_k[q] >= s_k[q])
#
# Sharding: data-parallel over batch. Core c owns batch c. The host groups
# queries by batch and routes each group's endpoint indices to the owning
# core; invalid queries (e < s) are routed to a zeroed pad row of h, so no
# masking pass is needed on device.
#
# On device, X @ W runs as a 3-term bf16 split (X = Xhi + Xlo, W = Whi + Wlo,
# h = Xhi Whi + Xhi Wlo + Xlo Whi accumulated in fp32 PSUM) which matches
# fp32 accuracy to ~5e-6 while using single-pass bf16 matmuls. The
# transposed stationary operand (X^T tiles) is produced by casting X to
# bf16 hi/lo in SBUF, spilling to DRAM, and DMA-transpose loading — no PE
# transposes. h spills to DRAM and endpoint rows are fetched with
# dma_gather, then written to compact per-core result buffers.
import numpy as np

B, S, D, P = 8, 2048, 1024, 256
NQ = 8192
NCORES = 8
C = 1280               # per-core query capacity (host falls back if exceeded)
CB = C // 128          # query blocks of 128
NIDX = 4 * C           # gather indices per core: s1 | e1 | s2 | e2
KB = D // 128          # contraction k-blocks
MB = S // 128          # row blocks of the batch slice
MCHUNK = 512           # m rows per transpose/matmul chunk
NMC = S // MCHUNK

ZROW = S               # h pad row index (zeroed): invalid/pad queries point here

_cache = {}


def _build_nc():
    import concourse.bacc as bacc
    import concourse.mybir as mybir
    import concourse.tile as tile

    f32 = mybir.dt.float32
    bf16 = mybir.dt.bfloat16
    nc = bacc.Bacc("TRN2", target_bir_lowering=False, debug=False,
                   num_devices=NCORES)

    x = nc.dram_tensor("x", [S, D], f32, kind="ExternalInput").ap()
    w = nc.dram_tensor("w", [D, P], f32, kind="ExternalInput").ap()
    bias = nc.dram_tensor("bias", [128, P], f32, kind="ExternalInput").ap()
    idx = nc.dram_tensor("idx", [128, NIDX // 16], mybir.dt.int16,
                         kind="ExternalInput").ap()
    r1 = nc.dram_tensor("r1", [C, 2 * P], f32, kind="ExternalOutput").ap()
    r2 = nc.dram_tensor("r2", [C, 2 * P], f32, kind="ExternalOutput").ap()

    with tile.TileContext(nc) as tc:
        with (
            tc.tile_pool(name="consts", bufs=1) as consts,
            tc.tile_pool(name="xin", bufs=3) as xin_pool,
            tc.tile_pool(name="xc", bufs=3) as xc_pool,
            tc.tile_pool(name="xt", bufs=2) as xt_pool,
            tc.tile_pool(name="hsb", bufs=3) as h_pool,
            tc.tile_pool(name="gath", bufs=1) as g_pool,
            tc.tile_pool(name="psh", bufs=4, space="PSUM") as psum_h_pool,
            tc.tile_pool(name="hdram", bufs=1, space="DRAM") as dram_pool,
        ):
            # weights: load fp32, split to bf16 hi/lo on DVE
            w_sb = consts.tile([128, KB, P], f32)
            nc.sync.dma_start(w_sb, w.rearrange("(kb k) p -> k kb p", k=128))
            whi = consts.tile([128, KB, P], bf16)
            wlo = consts.tile([128, KB, P], bf16)
            wtmp = consts.tile([128, KB, P], f32)
            nc.vector.tensor_copy(whi, w_sb)            # f32 -> bf16 (hi)
            nc.vector.tensor_copy(wtmp, whi)            # bf16 -> f32
            nc.vector.tensor_sub(wtmp, w_sb, wtmp)      # residual in f32
            nc.vector.tensor_copy(wlo, wtmp)            # f32 -> bf16 (lo)

            bias_sb = consts.tile([128, P], f32)
            nc.sync.dma_start(bias_sb, bias)
            idx_sb = consts.tile([128, NIDX // 16], mybir.dt.int16)
            nc.sync.dma_start(idx_sb, idx)

            h_dram = dram_pool.tile([S + 1, P], f32)
            xhi_dram = dram_pool.tile([S, D], bf16)
            xlo_dram = dram_pool.tile([S, D], bf16)

            # zero pad row of h (row ZROW) for invalid/padded queries
            zrow = consts.tile([1, P], f32)
            nc.vector.memset(zrow, 0.0)
            nc.sync.dma_start(h_dram[ZROW:ZROW + 1, :], zrow)

            # phase 1a: cast X to bf16 hi/lo, spill to DRAM
            for m in range(MB):
                x_sb = xin_pool.tile([128, D], f32, tag="x")
                nc.sync.dma_start(x_sb, x[m * 128:(m + 1) * 128, :])
                xhi = xc_pool.tile([128, D], bf16, tag="xhi")
                xlo = xc_pool.tile([128, D], bf16, tag="xlo")
                xtmp = xc_pool.tile([128, D], f32, tag="xtmp")
                nc.vector.tensor_copy(xhi, x_sb)
                nc.vector.tensor_copy(xtmp, xhi)
                nc.vector.tensor_sub(xtmp, x_sb, xtmp)
                nc.vector.tensor_copy(xlo, xtmp)
                nc.sync.dma_start(xhi_dram[m * 128:(m + 1) * 128, :], xhi)
                nc.sync.dma_start(xlo_dram[m * 128:(m + 1) * 128, :], xlo)

            # phase 1b: DMA-transpose loads + bf16 matmuls, one m-chunk at a
            # time; h = Xhi@Whi + Xhi@Wlo + Xlo@Whi (fp32 PSUM accumulate)
            for mc in range(NMC):
                rows = slice(mc * MCHUNK, (mc + 1) * MCHUNK)
                xhiT = xt_pool.tile([128, KB, MCHUNK], bf16, tag="xhiT")
                xloT = xt_pool.tile([128, KB, MCHUNK], bf16, tag="xloT")
                for kb in range(KB):
                    cols = slice(kb * 128, (kb + 1) * 128)
                    nc.sync.dma_start_transpose(xhiT[:, kb], xhi_dram[rows, cols])
                    nc.sync.dma_start_transpose(xloT[:, kb], xlo_dram[rows, cols])
                for t in range(MCHUNK // 128):
                    msl = slice(t * 128, (t + 1) * 128)
                    h_ps = psum_h_pool.tile([128, P], f32, tag="hps")
                    n_mm = 3 * KB
                    i_mm = 0
                    for kb in range(KB):
                        for lhsT, rhs in ((xhiT[:, kb, msl], whi[:, kb, :]),
                                          (xhiT[:, kb, msl], wlo[:, kb, :]),
                                          (xloT[:, kb, msl], whi[:, kb, :])):
                            nc.tensor.matmul(h_ps, lhsT, rhs,
                                             start=(i_mm == 0),
                                             stop=(i_mm == n_mm - 1))
                            i_mm += 1
                    h_sb = h_pool.tile([128, P], f32, tag="h")
                    nc.vector.tensor_add(h_sb, h_ps, bias_sb)
                    m = mc * (MCHUNK // 128) + t
                    nc.sync.dma_start(h_dram[m * 128:(m + 1) * 128, :], h_sb)

            # phase 2: gather endpoint rows, write compact results.
            # stream 0: s1 -> r1[:, :P]; 1: e1 -> r1[:, P:]; 2/3 likewise r2.
            CW = C // 16
            streams = [(r1, 0), (r1, P), (r2, 0), (r2, P)]
            for st, (r, col0) in enumerate(streams):
                g_sb = g_pool.tile([128, CB, P], f32, tag=f"g{st}")
                nc.gpsimd.dma_gather(
                    g_sb, h_dram[:, :], idx_sb[:, st * CW:(st + 1) * CW],
                    num_idxs=C, num_idxs_reg=C, elem_size=P,
                    single_packet=False)
                out_view = r.rearrange("(cb p) c -> p cb c", p=128)
                half = CB // 2
                for piece in range(2):
                    cbs = slice(piece * half, (piece + 1) * half)
                    nc.sync.dma_start(out_view[:, cbs, col0:col0 + P],
                                      g_sb[:, cbs, :])

    nc.compile()
    return nc


def _get_nc():
    if "nc" not in _cache:
        _cache["nc"] = _build_nc()
    return _cache["nc"]


def _numpy_ref(flag, encoded_input, start_ids_1, end_ids_1, query_batch_idx,
               start_ids_2, end_ids_2, W, b):
    h = encoded_input.astype(np.float32) @ W.astype(np.float32) + \
        b.astype(np.float32)
    qb = np.asarray(query_batch_idx).astype(np.int64)

    def span(s, e):
        s = np.asarray(s).astype(np.int64)
        e = np.asarray(e).astype(np.int64)
        rep = np.concatenate([h[qb, s], h[qb, e]], axis=-1)
        return rep * (e >= s)[:, None].astype(rep.dtype)

    return span(start_ids_1, end_ids_1), span(start_ids_2, end_ids_2)


def kernel(flag, encoded_input, start_ids_1, end_ids_1, query_batch_idx,
           start_ids_2, end_ids_2, W, b):
    from concourse.bass_utils import run_bass_kernel_spmd

    x_full = np.ascontiguousarray(np.asarray(encoded_input),
                                  dtype=np.float32)
    w_np = np.ascontiguousarray(np.asarray(W), dtype=np.float32)
    b_np = np.asarray(b).astype(np.float32)
    qb = np.asarray(query_batch_idx).astype(np.int64)
    s1 = np.asarray(start_ids_1).astype(np.int64)
    e1 = np.asarray(end_ids_1).astype(np.int64)
    s2 = np.asarray(start_ids_2).astype(np.int64)
    e2 = np.asarray(end_ids_2).astype(np.int64)

    perms = [np.nonzero(qb == bb)[0] for bb in range(B)]
    counts = [len(p) for p in perms]
    in_range = (qb.min() >= 0 and qb.max() < B and
                all(a.min() >= 0 and a.max() < S for a in (s1, e1, s2, e2)))
    if max(counts) > C or not in_range or x_full.shape != (B, S, D):
        res1, res2 = _numpy_ref(flag, x_full, s1, e1, qb, s2, e2, w_np, b_np)
        return np.asarray(res1, np.float32), np.asarray(res2, np.float32)

    bias_rep = np.ascontiguousarray(
        np.broadcast_to(b_np[None, :], (128, P)), dtype=np.float32)

    valid1 = e1 >= s1
    valid2 = e2 >= s2
    in_maps = []
    for bb in range(B):
        sel = perms[bb]
        n = counts[bb]
        st_arrs = []
        for a, v in ((s1, valid1), (e1, valid1), (s2, valid2), (e2, valid2)):
            ap = np.full(C, ZROW, np.int64)
            ap[:n] = np.where(v[sel], a[sel], ZROW)  # invalid -> zero row
            st_arrs.append(ap)
        idx_stream = np.concatenate(st_arrs).astype(np.int16)       # [4C]
        idx_w = idx_stream.reshape(NIDX // 16, 16).T                # [16, .]
        idx_w = np.ascontiguousarray(np.tile(idx_w, (8, 1)))        # [128, .]
        in_maps.append({
            "x": np.ascontiguousarray(x_full[bb]),
            "w": w_np,
            "bias": bias_rep,
            "idx": idx_w,
        })

    nc = _get_nc()
    out = run_bass_kernel_spmd(nc, in_maps, core_ids=list(range(NCORES)))
    _cache["last_run"] = out

    res1 = np.zeros((NQ, 2 * P), np.float32)
    res2 = np.zeros((NQ, 2 * P), np.float32)
    for bb in range(B):
        if counts[bb]:
            res1[perms[bb]] = out.results[bb]["r1"][:counts[bb]]
            res2[perms[bb]] = out.results[bb]["r2"][:counts[bb]]
    return res1, res2


# revision 5
# speedup vs baseline: 1.7256x; 1.7256x over previous
# Trainium2 Bass kernel for EndPointRepr (span endpoint representations).
#
# reference:
#   h = encoded_input @ W + b                    # [B, S, P]
#   res_k[q] = concat(h[qb[q], s_k[q]], h[qb[q], e_k[q]]) * (e_k[q] >= s_k[q])
#
# Sharding: data-parallel over batch. Core c owns batch c. The host groups
# queries by batch and routes each group's endpoint indices to the owning
# core; invalid queries (e < s) are routed to a zeroed pad row of h, so the
# validity mask costs nothing on device.
#
# Device pipeline (fp32 throughout):
#   phase 1: per 128-row block of the batch slice, PE-transpose X tiles
#            (k onto partitions), matmul against W k-tiles accumulating in
#            PSUM, add bias, spill h row-block to DRAM.
#   phase 2: dma_gather endpoint rows of h into SBUF, DMA compact per-core
#            [C, 2P] result buffers back to DRAM.
# Host scatters per-core buffers into the full [NQ, 2P] outputs.
import numpy as np

B, S, D, P = 8, 2048, 1024, 256
NQ = 8192
NCORES = 8
C = 1280               # per-core query capacity (host falls back if exceeded)
CB = C // 128          # query blocks of 128
NIDX = 4 * C           # gather indices per core: s1 | e1 | s2 | e2
KB = D // 128          # contraction k-blocks
MB = S // 128          # row blocks of the batch slice

ZROW = S               # h pad row (zeroed): invalid/pad queries point here

_cache = {}


def _build_nc():
    import concourse.bacc as bacc
    import concourse.mybir as mybir
    import concourse.tile as tile
    from concourse.masks import make_identity

    f32 = mybir.dt.float32
    nc = bacc.Bacc("TRN2", target_bir_lowering=False, debug=False,
                   num_devices=NCORES)

    x = nc.dram_tensor("x", [S, D], f32, kind="ExternalInput").ap()
    w = nc.dram_tensor("w", [D, P], f32, kind="ExternalInput").ap()
    bias = nc.dram_tensor("bias", [128, P], f32, kind="ExternalInput").ap()
    idx = nc.dram_tensor("idx", [128, NIDX // 16], mybir.dt.int16,
                         kind="ExternalInput").ap()
    r1 = nc.dram_tensor("r1", [C, 2 * P], f32, kind="ExternalOutput").ap()
    r2 = nc.dram_tensor("r2", [C, 2 * P], f32, kind="ExternalOutput").ap()

    with tile.TileContext(nc) as tc:
        with (
            tc.tile_pool(name="consts", bufs=1) as consts,
            tc.tile_pool(name="xin", bufs=4) as xin_pool,
            tc.tile_pool(name="xt", bufs=8) as xt_pool,
            tc.tile_pool(name="hsb", bufs=4) as h_pool,
            tc.tile_pool(name="gath", bufs=1) as g_pool,
            tc.tile_pool(name="pst", bufs=4, space="PSUM") as psum_t_pool,
            tc.tile_pool(name="psh", bufs=3, space="PSUM") as psum_h_pool,
            tc.tile_pool(name="hdram", bufs=1, space="DRAM") as dram_pool,
        ):
            identity = consts.tile([128, 128], f32)
            make_identity(nc, identity)

            w_sb = consts.tile([128, KB, P], f32)
            nc.sync.dma_start(w_sb, w.rearrange("(kb k) p -> k kb p", k=128))
            bias_sb = consts.tile([128, P], f32)
            nc.sync.dma_start(bias_sb, bias)
            idx_sb = consts.tile([128, NIDX // 16], mybir.dt.int16)
            nc.sync.dma_start(idx_sb, idx)

            h_dram = dram_pool.tile([S + 1, P], f32)
            zrow = consts.tile([1, P], f32)
            nc.vector.memset(zrow, 0.0)
            nc.sync.dma_start(h_dram[ZROW:ZROW + 1, :], zrow)

            # h = X @ W + b, one [128, P] row-block at a time
            for m in range(MB):
                x_sb = xin_pool.tile([128, D], f32, tag="x")
                nc.sync.dma_start(x_sb, x[m * 128:(m + 1) * 128, :])
                h_ps = psum_h_pool.tile([128, P], f32, tag="hps")
                for kb2 in range(KB // 2):
                    xt_ps = psum_t_pool.tile([128, 2, 128], f32, tag="xtps")
                    for j in range(2):
                        kb = 2 * kb2 + j
                        nc.tensor.transpose(
                            xt_ps[:, j], x_sb[:, kb * 128:(kb + 1) * 128],
                            identity)
                    xt_sb = xt_pool.tile([128, 2, 128], f32, tag="xt")
                    # alternate evac engine so DVE and ACT share the load
                    if kb2 % 2 == 0:
                        nc.vector.tensor_copy(xt_sb, xt_ps)
                    else:
                        nc.scalar.copy(xt_sb, xt_ps)
                    for j in range(2):
                        kb = 2 * kb2 + j
                        nc.tensor.matmul(h_ps, xt_sb[:, j], w_sb[:, kb, :],
                                         start=(kb == 0), stop=(kb == KB - 1))
                h_sb = h_pool.tile([128, P], f32, tag="h")
                nc.vector.tensor_add(h_sb, h_ps, bias_sb)
                nc.sync.dma_start(h_dram[m * 128:(m + 1) * 128, :], h_sb)

            # gather endpoint rows; stream layout: s1 | e1 | s2 | e2
            CW = C // 16
            streams = [(r1, 0), (r1, P), (r2, 0), (r2, P)]
            for st, (r, col0) in enumerate(streams):
                g_sb = g_pool.tile([128, CB, P], f32, tag=f"g{st}")
                nc.gpsimd.dma_gather(
                    g_sb, h_dram[:, :], idx_sb[:, st * CW:(st + 1) * CW],
                    num_idxs=C, num_idxs_reg=C, elem_size=P,
                    single_packet=False)
                out_view = r.rearrange("(cb p) c -> p cb c", p=128)
                half = CB // 2
                for piece in range(2):
                    cbs = slice(piece * half, (piece + 1) * half)
                    nc.sync.dma_start(out_view[:, cbs, col0:col0 + P],
                                      g_sb[:, cbs, :])

    nc.compile()
    return nc


def _get_nc():
    if "nc" not in _cache:
        _cache["nc"] = _build_nc()
    return _cache["nc"]


def _numpy_ref(flag, encoded_input, start_ids_1, end_ids_1, query_batch_idx,
               start_ids_2, end_ids_2, W, b):
    h = encoded_input.astype(np.float32) @ W.astype(np.float32) + \
        b.astype(np.float32)
    qb = np.asarray(query_batch_idx).astype(np.int64)

    def span(s, e):
        s = np.asarray(s).astype(np.int64)
        e = np.asarray(e).astype(np.int64)
        rep = np.concatenate([h[qb, s], h[qb, e]], axis=-1)
        return rep * (e >= s)[:, None].astype(rep.dtype)

    return span(start_ids_1, end_ids_1), span(start_ids_2, end_ids_2)


def kernel(flag, encoded_input, start_ids_1, end_ids_1, query_batch_idx,
           start_ids_2, end_ids_2, W, b):
    from concourse.bass_utils import run_bass_kernel_spmd

    x_full = np.ascontiguousarray(np.asarray(encoded_input),
                                  dtype=np.float32)
    w_np = np.ascontiguousarray(np.asarray(W), dtype=np.float32)
    b_np = np.asarray(b).astype(np.float32)
    qb = np.asarray(query_batch_idx).astype(np.int64)
    s1 = np.asarray(start_ids_1).astype(np.int64)
    e1 = np.asarray(end_ids_1).astype(np.int64)
    s2 = np.asarray(start_ids_2).astype(np.int64)
    e2 = np.asarray(end_ids_2).astype(np.int64)

    perms = [np.nonzero(qb == bb)[0] for bb in range(B)]
    counts = [len(p) for p in perms]
    in_range = (qb.min() >= 0 and qb.max() < B and
                all(a.min() >= 0 and a.max() < S for a in (s1, e1, s2, e2)))
    if max(counts) > C or not in_range or x_full.shape != (B, S, D):
        res1, res2 = _numpy_ref(flag, x_full, s1, e1, qb, s2, e2, w_np, b_np)
        return np.asarray(res1, np.float32), np.asarray(res2, np.float32)

    bias_rep = np.ascontiguousarray(
        np.broadcast_to(b_np[None, :], (128, P)), dtype=np.float32)

    valid1 = e1 >= s1
    valid2 = e2 >= s2
    in_maps = []
    for bb in range(B):
        sel = perms[bb]
        n = counts[bb]
        st_arrs = []
        for a, v in ((s1, valid1), (e1, valid1), (s2, valid2), (e2, valid2)):
            ap = np.full(C, ZROW, np.int64)
            ap[:n] = np.where(v[sel], a[sel], ZROW)
            st_arrs.append(ap)
        idx_stream = np.concatenate(st_arrs).astype(np.int16)       # [4C]
        idx_w = idx_stream.reshape(NIDX // 16, 16).T                # [16, .]
        idx_w = np.ascontiguousarray(np.tile(idx_w, (8, 1)))        # [128, .]
        in_maps.append({
            "x": np.ascontiguousarray(x_full[bb]),
            "w": w_np,
            "bias": bias_rep,
            "idx": idx_w,
        })

    nc = _get_nc()
    out = run_bass_kernel_spmd(nc, in_maps, core_ids=list(range(NCORES)))
    _cache["last_run"] = out

    res1 = np.zeros((NQ, 2 * P), np.float32)
    res2 = np.zeros((NQ, 2 * P), np.float32)
    for bb in range(B):
        if counts[bb]:
            res1[perms[bb]] = out.results[bb]["r1"][:counts[bb]]
            res2[perms[bb]] = out.results[bb]["r2"][:counts[bb]]
    return res1, res2
